# revision 18
# baseline (speedup 1.0000x reference)
"""Multi-Head Latent Attention (MLA) Bass kernel for 8 Trainium2 NeuronCores.

Sharding:
  - latent projections (d_kv, d_q): sequence-sharded (BS/8 rows per core),
    per-latent AllGathers replicate the latents (q-latent gathered in fp8).
  - up-projections (u_k, u_v, u_q, qr) + attention: head-sharded, 2 heads/core.
  - context: AllToAll re-shards to sequence; out_proj sequence-parallel.
Structure (v2):
  - dummy 128B AllGather at t=0 absorbs the cross-device rendezvous barrier
    while phase A computes.
  - attention computes scoresT [sk, sq] then ctxT [d, sq] with lhsT = V
    (N=512 moving ops); softmax denominators via DVE pairwise adds +
    ones-vector column-sum matmuls + rank-1 PE broadcast; normalization is a
    single DVE multiply per head-block. No PE transposes.
  - q-side projections (pass-2) are interleaved into the attention block loop
    to fill tensor-engine slack under the scalar-engine exp stream.
  - rope/V biases are zeros by problem spec and are not injected on device;
    out_b is added on host.
"""
import sys
import os

for _p in ("/opt/trn_rl_repo", "/root/.axon_site/_ro/trn_rl_repo"):
    if os.path.isdir(_p) and _p not in sys.path:
        sys.path.insert(0, _p)

import math
import numpy as np
import ml_dtypes

BF = ml_dtypes.bfloat16

import concourse.bacc as bacc
import concourse.mybir as mybir
from concourse import tile
from concourse.bass_utils import run_bass_kernel_spmd

# problem dims (hardcoded)
B, S, H, Dh, Dr, HID, C = 2, 2048, 16, 128, 64, 2048, 512
BS = B * S                      # 4096
NCORES = 8
H_LOC = H // NCORES             # 2
S_LOC = BS // NCORES            # 512
SCALE = 1.0 / math.sqrt(Dh + Dr)

F32 = mybir.dt.float32
F32R = mybir.dt.float32r
F16 = mybir.dt.float16
BF16 = mybir.dt.bfloat16
F8 = mybir.dt.float8e4

_CACHE = {}


def _build_program():
    nc = bacc.Bacc("TRN2", target_bir_lowering=False, debug=False,
                   num_devices=NCORES)

    xT_in = nc.dram_tensor("xT_loc", [HID, S_LOC], BF16, kind="ExternalInput")
    wdkvT = nc.dram_tensor("wdkvT", [HID, C], BF16, kind="ExternalInput")
    wdqT = nc.dram_tensor("wdqT", [HID, C], BF16, kind="ExternalInput")
    bdkv = nc.dram_tensor("bdkv", [4, 128, 1], F32, kind="ExternalInput")
    bdq = nc.dram_tensor("bdq", [4, 128, 1], F32, kind="ExternalInput")
    ukT = nc.dram_tensor("ukT", [C, 256], BF16, kind="ExternalInput")
    uqT = nc.dram_tensor("uqT", [C, 256], BF16, kind="ExternalInput")
    uvT = nc.dram_tensor("uvT", [C, 256], BF16, kind="ExternalInput")
    buk = nc.dram_tensor("buk", [2, 128, 1], F32, kind="ExternalInput")
    buq = nc.dram_tensor("buq", [2, 128, 1], F32, kind="ExternalInput")
    wrAq = nc.dram_tensor("wrAq", [C, 128], BF16, kind="ExternalInput")
    wrBq = nc.dram_tensor("wrBq", [C, 128], BF16, kind="ExternalInput")
    wrAk = nc.dram_tensor("wrAk", [C, 128], BF16, kind="ExternalInput")
    wrBk = nc.dram_tensor("wrBk", [C, 128], BF16, kind="ExternalInput")
    c1_in = nc.dram_tensor("c1", [128, S], F16, kind="ExternalInput")
    c2_in = nc.dram_tensor("c2", [128, S], F16, kind="ExternalInput")

    woT = nc.dram_tensor("woT", [H * Dh, HID], BF16, kind="ExternalInput")
    out_sl = nc.dram_tensor("out_slice", [S_LOC, HID], F32, kind="ExternalOutput")

    groups = [list(range(NCORES))]
    EXP = mybir.ActivationFunctionType.Exp
    IDENT = mybir.ActivationFunctionType.Identity

    with tile.TileContext(nc) as tc:
        with tc.tile_pool(name="dram", bufs=1, space="DRAM") as dram:
            kv_ag_i = dram.tile([4, 128, S_LOC], BF16)
            kv_ag_o = dram.tile([NCORES, 4, 128, S_LOC], BF16,
                                addr_space="Shared", name="kvago")
            ql_ag_i = dram.tile([4, 128, S_LOC], BF16)
            ql_ag_o = dram.tile([NCORES, 4, 128, S_LOC], BF16,
                                addr_space="Shared", name="qlago")
            a2a_i = dram.tile([NCORES, H_LOC * Dh, S_LOC], BF16)
            a2a_o = dram.tile([NCORES, H_LOC * Dh, S_LOC], BF16,
                              name="a2ao")

            with tc.tile_pool(name="const", bufs=1) as const:
                ones_col = const.tile([128, 1], BF16)
                nc.vector.memset(ones_col[:], 1.0)
                ones_f32 = const.tile([1, 128], F32)
                nc.vector.memset(ones_f32[:], 1.0)
                ones_bc = const.tile([1, 128], F32R)
                with nc.allow_low_precision(reason="exact 1.0 constant"):
                    nc.vector.tensor_copy(ones_bc[:], ones_f32[:])

                # ============ Phase A: latents (sequence-sharded) ============
                with tc.tile_pool(name="phA", bufs=1) as phA, \
                     tc.tile_pool(name="phAb", bufs=4) as phAb, \
                     tc.tile_pool(name="psA", bufs=4, space="PSUM") as psA:
                    xts, wkv, wql = [], [], []
                    for ht in range(16):
                        xt = phA.tile([128, S_LOC], BF16, tag=f"xt{ht}",
                                      name=f"xt{ht}")
                        nc.sync.dma_start(out=xt[:],
                                          in_=xT_in[ht * 128:(ht + 1) * 128, :])
                        xts.append(xt)
                        w = phA.tile([128, C], BF16, tag=f"wk{ht}",
                                     name=f"wk{ht}")
                        nc.sync.dma_start(out=w[:],
                                          in_=wdkvT[ht * 128:(ht + 1) * 128, :])
                        wkv.append(w)
                    for ht in range(16):
                        w = phA.tile([128, C], BF16, tag=f"wq{ht}",
                                     name=f"wq{ht}")
                        nc.sync.dma_start(out=w[:],
                                          in_=wdqT[ht * 128:(ht + 1) * 128, :])
                        wql.append(w)
                    for li, (wfull, bdram, agi, ago, ldt) in enumerate(
                            ((wkv, bdkv, kv_ag_i, kv_ag_o, BF16),
                             (wql, bdq, ql_ag_i, ql_ag_o, BF16))):
                        for ct in range(4):
                            ps = psA.tile([128, S_LOC], F32, tag="ps", name="psa")
                            for ht in range(16):
                                nc.tensor.matmul(
                                    ps[:],
                                    wfull[ht][:, ct * 128:(ct + 1) * 128],
                                    xts[ht][:],
                                    start=(ht == 0), stop=(ht == 15))
                            bt = phAb.tile([128, 1], F32, tag="blat", name="blat")
                            nc.sync.dma_start(out=bt[:], in_=bdram[ct])
                            lat = phAb.tile([128, S_LOC], ldt, tag=f"lat{li}",
                                            bufs=3, name=f"lat{li}")
                            nc.vector.tensor_scalar_add(lat[:], ps[:], bt[:])
                            nc.sync.dma_start(out=agi[ct], in_=lat[:])
                        nc.gpsimd.collective_compute(
                            "AllGather", mybir.AluOpType.bypass,
                            replica_groups=groups,
                            ins=[agi.opt()], outs=[ago.opt()])

                # ====== persistent attention operand tiles (phases B-C) ======
                with tc.tile_pool(name="attn", bufs=1) as attn:
                    kcT = [attn.tile([128, BS], BF16, tag=f"kcT{h}",
                                     name=f"kcT{h}") for h in range(2)]
                    qcT = [attn.tile([128, BS], BF16, tag=f"qcT{h}",
                                     name=f"qcT{h}") for h in range(2)]
                    krT = attn.tile([128, BS], BF16, tag="krT", name="krT")
                    qrT = attn.tile([128, BS], BF16, tag="qrT", name="qrT")
                    v_sb = attn.tile([128, 32, 256], BF16, tag="v", name="v_sb")
                    c1s = attn.tile([128, S], F16, tag="c1", name="c1s")
                    nc.sync.dma_start(out=c1s[:], in_=c1_in[:])
                    c2s = attn.tile([128, S], F16, tag="c2", name="c2s")
                    nc.sync.dma_start(out=c2s[:], in_=c2_in[:])

                    # ========= pass 1: kv-dependent (k_c, rope-k, V) =========
                    with tc.tile_pool(name="phB", bufs=1) as phB, \
                         tc.tile_pool(name="phBt", bufs=2) as phBt, \
                         tc.tile_pool(name="psB", bufs=2, space="PSUM") as psB:
                        upw = {}
                        for nm, t, w_ in (("uk", ukT, 256), ("uv", uvT, 256),
                                          ("wrAk", wrAk, 128),
                                          ("wrBk", wrBk, 128)):
                            tl = []
                            for ct in range(4):
                                wt = phB.tile([128, w_], BF16, tag=f"{nm}{ct}",
                                              name=f"{nm}{ct}")
                                nc.sync.dma_start(
                                    out=wt[:],
                                    in_=t[ct * 128:(ct + 1) * 128, :])
                                tl.append(wt)
                            upw[nm] = tl
                        buk_t = []
                        for h in range(2):
                            bt_ = phB.tile([128, 1], F32, tag=f"buk{h}",
                                           name=f"buk{h}")
                            nc.sync.dma_start(out=bt_[:], in_=buk[h])
                            buk_t.append(bt_)

                        # keep the PE clock un-throttled across the AG-kv
                        # wait: write-only filler matmuls on resident weights
                        # (results never read).
                        psj = psB.tile([128, 256], F32, tag="junk",
                                       bufs=1, name="psj")
                        for _ in range(300):
                            nc.tensor.matmul(psj[:],
                                             upw["uk"][0][:, 0:128],
                                             upw["uv"][0][:],
                                             start=True, stop=True)

                        for j2 in range(8):
                            sl = slice(j2 * 512, (j2 + 1) * 512)
                            pos = slice((j2 % 4) * 512, (j2 % 4) * 512 + 512)
                            kv_sb = []
                            for ct in range(4):
                                kt_ = phBt.tile([128, 512], BF16,
                                                tag=f"kv{j2}_{ct}", bufs=1,
                                                name=f"kv{j2}_{ct}")
                                nc.scalar.dma_start(out=kt_[:],
                                                      in_=kv_ag_o[j2, ct])
                                kv_sb.append(kt_)
                            for h in range(2):
                                hc = slice(h * 128, (h + 1) * 128)
                                ps = psB.tile([128, 512], F32, tag="psKC",
                                              name="pskc")
                                for ct in range(4):
                                    nc.tensor.matmul(ps[:],
                                                     upw["uk"][ct][:, hc],
                                                     kv_sb[ct][:],
                                                     start=(ct == 0),
                                                     stop=(ct == 3))
                                nc.scalar.activation(
                                    kcT[h][:, sl], ps[:], IDENT,
                                    bias=buk_t[h][:])
                            psa_ = psB.tile([128, 512], F32, tag="psRA",
                                            bufs=1, name="psra")
                            psb_ = psB.tile([128, 512], F32, tag="psRB",
                                            bufs=1, name="psrb")
                            for ct in range(4):
                                nc.tensor.matmul(psa_[:], upw["wrAk"][ct][:],
                                                 kv_sb[ct][:],
                                                 start=(ct == 0), stop=(ct == 3))
                            for ct in range(4):
                                nc.tensor.matmul(psb_[:], upw["wrBk"][ct][:],
                                                 kv_sb[ct][:],
                                                 start=(ct == 0), stop=(ct == 3))
                            t1 = phBt.tile([128, 512], F32, tag="t1", bufs=2,
                                           name="t1")
                            nc.vector.tensor_mul(t1[:], psa_[:], c1s[:, pos])
                            t2 = phBt.tile([128, 512], F32, tag="t2", bufs=2,
                                           name="t2")
                            nc.vector.tensor_mul(t2[:], psb_[:], c2s[:, pos])
                            nc.vector.tensor_add(krT[:, sl], t1[:], t2[:])
                            for ss in range(4):
                                psv_ = psB.tile([128, 256], F32, tag="psV",
                                                name="psv")
                                ssl = slice(ss * 128, (ss + 1) * 128)
                                for ct in range(4):
                                    nc.tensor.matmul(psv_[:],
                                                     kv_sb[ct][:, ssl],
                                                     upw["uv"][ct][:],
                                                     start=(ct == 0),
                                                     stop=(ct == 3))
                                st = j2 * 4 + ss
                                nc.vector.tensor_copy(v_sb[:, st, :], psv_[:])

                    # ====== persistent q-side weights (used inside C) ======
                    uqw, wrq = [], {}
                    for ct in range(4):
                        wt = attn.tile([128, 256], BF16, tag=f"uq{ct}",
                                       name=f"uq{ct}")
                        nc.sync.dma_start(out=wt[:],
                                          in_=uqT[ct * 128:(ct + 1) * 128, :])
                        uqw.append(wt)
                    for nm, t in (("wrAq", wrAq), ("wrBq", wrBq)):
                        tl = []
                        for ct in range(4):
                            wt = attn.tile([128, 128], BF16, tag=f"{nm}{ct}",
                                           name=f"{nm}{ct}")
                            nc.sync.dma_start(
                                out=wt[:], in_=t[ct * 128:(ct + 1) * 128, :])
                            tl.append(wt)
                        wrq[nm] = tl
                    buq_t = []
                    for h in range(2):
                        bt_ = attn.tile([128, 1], F32, tag=f"buq{h}",
                                        name=f"buq{h}")
                        nc.sync.dma_start(out=bt_[:], in_=buq[h])
                        buq_t.append(bt_)

                    # prefetch out-proj weights for the first output column
                    # block so phase D starts without a DMA stall.
                    with tc.tile_pool(name="phDw0", bufs=1) as phDw0:
                        wos0 = []
                        for dht in range(16):
                            wo = phDw0.tile([128, 512], BF16, tag=f"wo0{dht}",
                                            name=f"wo0{dht}")
                            nc.scalar.dma_start(
                                out=wo[:],
                                in_=woT[dht * 128:(dht + 1) * 128, 0:512])
                            wos0.append(wo)

                        # ================= Phase C: attention =================
                        with tc.tile_pool(name="phC", bufs=1) as phC, \
                             tc.tile_pool(name="psC", bufs=1,
                                          space="PSUM") as psC:
                            def pass2(g):
                                """q-side projections for seq block g."""
                                if g >= 8:
                                    return
                                sl = slice(g * 512, (g + 1) * 512)
                                pos = slice((g % 4) * 512, (g % 4) * 512 + 512)
                                ql_sb = []
                                for ct in range(4):
                                    qs = phC.tile([128, 512], BF16,
                                                  tag=f"qs{ct}", bufs=2,
                                                  name=f"qs{ct}")
                                    nc.scalar.dma_start(
                                        out=qs[:], in_=ql_ag_o[g, ct])
                                    ql_sb.append(qs)
                                psq = psC.tile([128, 2, 512], F32, tag="ps2",
                                               bufs=2, name="psq")
                                for h in range(2):
                                    hc = slice(h * 128, (h + 1) * 128)
                                    for ct in range(4):
                                        nc.tensor.matmul(psq[:, h, :],
                                                         uqw[ct][:, hc],
                                                         ql_sb[ct][:],
                                                         start=(ct == 0),
                                                         stop=(ct == 3))
                                for h in range(2):
                                    nc.vector.tensor_scalar_add(
                                        qcT[h][:, sl], psq[:, h, :],
                                        buq_t[h][:])
                                psr = psC.tile([128, 2, 512], F32, tag="ps2",
                                               bufs=2, name="psr")
                                for ct in range(4):
                                    nc.tensor.matmul(psr[:, 0, :],
                                                     wrq["wrAq"][ct][:],
                                                     ql_sb[ct][:],
                                                     start=(ct == 0),
                                                     stop=(ct == 3))
                                for ct in range(4):
                                    nc.tensor.matmul(psr[:, 1, :],
                                                     wrq["wrBq"][ct][:],
                                                     ql_sb[ct][:],
                                                     start=(ct == 0),
                                                     stop=(ct == 3))
                                t1 = phC.tile([128, 512], F32, tag="t1",
                                              bufs=1, name="t1c")
                                nc.vector.tensor_mul(t1[:], psr[:, 0, :],
                                                     c1s[:, pos])
                                t2 = phC.tile([128, 512], F32, tag="t2",
                                              bufs=1, name="t2c")
                                nc.vector.tensor_mul(t2[:], psr[:, 1, :],
                                                     c2s[:, pos])
                                nc.vector.tensor_add(qrT[:, sl], t1[:], t2[:])

                            def denom_dve(probs_p):
                                """bf16 pairwise-add tree over sk tiles."""
                                p8 = phC.tile([128, 2, 8, 512], BF16,
                                              tag="p8", bufs=1, name="p8")
                                for h in range(2):
                                    nc.vector.tensor_add(
                                        p8[:, h, :, :],
                                        probs_p[:, 0:8, h, :],
                                        probs_p[:, 8:16, h, :])
                                p4 = phC.tile([128, 2, 4, 512], BF16,
                                              tag="p4", bufs=1, name="p4")
                                for h in range(2):
                                    nc.vector.tensor_add(
                                        p4[:, h, :, :],
                                        p8[:, h, 0:4, :],
                                        p8[:, h, 4:8, :])
                                return p4

                            def denom_pe(p4, pe_bcast=False):
                                """column-sum + reciprocal + broadcast."""
                                dsum = psC.tile([128, 2, 512], F32,
                                                tag="dsum", bufs=1,
                                                name="dsum")
                                for h in range(2):
                                    for i in range(4):
                                        nc.tensor.matmul(
                                            dsum[0:1, h, :], ones_col[:],
                                            p4[:, h, i, :],
                                            start=(i == 0), stop=(i == 3))
                                rec = phC.tile([1, 2, 512], F32R, tag="rec",
                                               bufs=1, name="rec")
                                with nc.allow_low_precision(
                                        reason="f32r keeps ~13 mantissa "
                                        "bits; ample for softmax "
                                        "denominators"):
                                    for h in range(2):
                                        nc.vector.reciprocal(rec[0:1, h, :],
                                                             dsum[0:1, h, :])
                                recb = phC.tile([128, 2, 512], F32R,
                                                tag="recb", bufs=1,
                                                name="recb")
                                if pe_bcast:
                                    for h in range(2):
                                        nc.tensor.matmul(dsum[:, h, :],
                                                         ones_bc[:],
                                                         rec[0:1, h, :],
                                                         start=True,
                                                         stop=True)
                                    nc.vector.tensor_copy(recb[:], dsum[:])
                                else:
                                    nc.gpsimd.partition_broadcast(
                                        recb[:], rec[0:1, :, :])
                                return recb

                            def finish_stg(prev, psvs, recb):
                                pg, pb, probs_p = prev
                                for h in range(2):
                                    stg = phC.tile([128, 512], BF16,
                                                   tag=f"stg{h}", bufs=1,
                                                   name=f"stg{h}")
                                    nc.vector.tensor_mul(stg[:], psvs[h][:],
                                                         recb[:, h, :])
                                    nc.sync.dma_start(
                                        out=a2a_i[pg,
                                                  h * 128:(h + 1) * 128, :],
                                        in_=stg[:])

                            pass2(0)
                            pass2(1)
                            # software pipeline: block g's scores+exp stream
                            # carries block g-1's PV + denominator matmuls so
                            # the tensor engine never drains behind the
                            # scalar-engine exp chain. Column-sums for g-1 are
                            # issued right after the second score group so the
                            # reciprocal/broadcast chain retires mid-iteration.
                            prev = None
                            for b in range(2):
                                for sqb in range(4):
                                    g = b * 4 + sqb
                                    qsl = slice(b * S + sqb * 512,
                                                b * S + sqb * 512 + 512)
                                    probs = phC.tile([128, 16, 2, 512], BF16,
                                                     tag="probs", bufs=2,
                                                     name="probs")
                                    if prev is not None:
                                        p4p = denom_dve(prev[2])
                                        psvs = [psC.tile([128, 512], F32,
                                                         tag=f"psv{h}",
                                                         bufs=1,
                                                         name=f"psv{h}")
                                                for h in range(2)]
                                    recb = None
                                    for skt in range(16):
                                        ps2 = psC.tile([128, 2, 512], F32,
                                                       tag="ps2", bufs=2,
                                                       name="ps2")
                                        ksl = slice(b * S + skt * 128,
                                                    b * S + skt * 128 + 128)
                                        for h in range(2):
                                            nc.tensor.matmul(
                                                ps2[:, h, :],
                                                kcT[h][:, ksl],
                                                qcT[h][:, qsl],
                                                start=True, stop=False)
                                        for h in range(2):
                                            hr = slice(h * 64, h * 64 + 64)
                                            nc.tensor.matmul(
                                                ps2[:, h, :],
                                                krT[hr, ksl],
                                                qrT[hr, qsl],
                                                start=False, stop=True)
                                        nc.scalar.activation(
                                            probs[:, skt, :, :], ps2[:], EXP)
                                        if prev is not None:
                                            pg, pb, probs_p = prev
                                            for h in range(2):
                                                nc.tensor.matmul(
                                                    psvs[h][:],
                                                    v_sb[:, pb * 16 + skt,
                                                         h * 128:
                                                         (h + 1) * 128],
                                                    probs_p[:, skt, h, :],
                                                    start=(skt == 0),
                                                    stop=(skt == 15))
                                            if skt == 5:
                                                recb = denom_pe(p4p)
                                    if prev is not None:
                                        finish_stg(prev, psvs, recb)
                                    pass2(g + 2)
                                    prev = (g, b, probs)
                            # drain the last block
                            pg, pb, probs_p = prev
                            p4p = denom_dve(probs_p)
                            psvs = [psC.tile([128, 512], F32, tag=f"psv{h}",
                                             bufs=1, name=f"psv{h}")
                                    for h in range(2)]
                            for skt in range(16):
                                for h in range(2):
                                    nc.tensor.matmul(
                                        psvs[h][:],
                                        v_sb[:, pb * 16 + skt,
                                             h * 128:(h + 1) * 128],
                                        probs_p[:, skt, h, :],
                                        start=(skt == 0), stop=(skt == 15))
                            finish_stg(prev, psvs,
                                       denom_pe(p4p, pe_bcast=True))
                            nc.gpsimd.collective_compute(
                                "AllToAll", mybir.AluOpType.bypass,
                                replica_groups=groups,
                                ins=[a2a_i.opt()], outs=[a2a_o.opt()])

                        # ============== Phase D: out projection ==============
                        with tc.tile_pool(name="phD", bufs=1) as phD, \
                             tc.tile_pool(name="phDw", bufs=2) as phDw, \
                             tc.tile_pool(name="phDo", bufs=3) as phDo, \
                             tc.tile_pool(name="psD", bufs=2,
                                          space="PSUM") as psD:
                            psj2 = psD.tile([128, 512], F32, tag="junk",
                                            bufs=1, name="psj2")
                            for _ in range(180):
                                nc.tensor.matmul(psj2[:],
                                                 wos0[0][:, 0:128],
                                                 wos0[1][:],
                                                 start=True, stop=True)
                            csl = []
                            for dht in range(16):
                                cf = phD.tile([128, S_LOC], BF16,
                                              tag=f"cf{dht}", name=f"cf{dht}")
                                nc.sync.dma_start(
                                    out=cf[:],
                                    in_=a2a_o[dht // 2,
                                              (dht % 2) * 128:
                                              (dht % 2) * 128 + 128, :])
                                csl.append(cf)
                            for ot in range(4):
                                osl = slice(ot * 512, (ot + 1) * 512)
                                if ot == 0:
                                    wos = wos0
                                else:
                                    wos = []
                                    for dht in range(16):
                                        wo = phDw.tile([128, 512], BF16,
                                                       tag=f"wo{dht}",
                                                       name=f"wo{dht}")
                                        nc.sync.dma_start(
                                            out=wo[:],
                                            in_=woT[dht * 128:(dht + 1) * 128,
                                                    osl])
                                        wos.append(wo)
                                for ssub in range(4):
                                    pso = psD.tile([128, 512], F32, tag="psO",
                                                   name="pso")
                                    ssl = slice(ssub * 128, (ssub + 1) * 128)
                                    for dht in range(16):
                                        nc.tensor.matmul(pso[:],
                                                         csl[dht][:, ssl],
                                                         wos[dht][:],
                                                         start=(dht == 0),
                                                         stop=(dht == 15))
                                    osb = phDo.tile([128, 512], F32,
                                                    tag="osb", name="osb")
                                    nc.vector.tensor_copy(osb[:], pso[:])
                                    nc.sync.dma_start(out=out_sl[ssl, osl],
                                                      in_=osb[:])

    nc.compile()
    return nc


def _host_prep(inputs):
    """Build per-core input maps from the full problem inputs."""
    x = np.asarray(inputs["x"], np.float32)
    xT = np.ascontiguousarray(x.reshape(BS, HID).T)            # [HID, BS]
    wdkvT = np.ascontiguousarray(np.asarray(inputs["d_kv_w"], np.float32).T.astype(BF))
    wdqT = np.ascontiguousarray(np.asarray(inputs["d_q_w"], np.float32).T.astype(BF))
    bdkv = np.asarray(inputs["d_kv_b"], np.float32).reshape(4, 128, 1)
    bdq = np.asarray(inputs["d_q_b"], np.float32).reshape(4, 128, 1)

    uk3 = np.asarray(inputs["u_k_w"], np.float32).reshape(H, Dh, C)
    uq3 = np.asarray(inputs["u_q_w"], np.float32).reshape(H, Dh, C) * SCALE
    uv3 = np.asarray(inputs["u_v_w"], np.float32).reshape(H, Dh, C)
    buk2 = np.asarray(inputs["u_k_b"], np.float32).reshape(H, Dh)
    buq2 = np.asarray(inputs["u_q_b"], np.float32).reshape(H, Dh) * SCALE
    qr3 = np.asarray(inputs["qr_w"], np.float32).reshape(H, Dr, C)

    # rope tables (positions 0..S-1)
    i32 = np.arange(32, dtype=np.float32)
    inv_freq = (10000.0 ** (-(2.0 * i32) / Dr)).astype(np.float32)  # [32]
    pos = np.arange(S, dtype=np.float32)
    ang = pos[None, :] * inv_freq[:, None]                     # [32, S]
    cos, sin = np.cos(ang), np.sin(ang)
    c1 = np.concatenate([cos, sin, cos, sin], 0).astype(np.float32)
    c2 = np.concatenate([-sin, cos, -sin, cos], 0).astype(np.float32)

    woT = np.ascontiguousarray(np.asarray(inputs["out_w"], np.float32).T.astype(BF))

    in_maps = []
    for j in range(NCORES):
        hs = [2 * j, 2 * j + 1]
        ukT_l = uk3[hs].transpose(2, 0, 1).reshape(C, 256)
        uqT_l = uq3[hs].transpose(2, 0, 1).reshape(C, 256)
        uvT_l = uv3[hs].transpose(2, 0, 1).reshape(C, 256)
        we = [qr3[h, 0::2, :] for h in hs]    # [32, C] each
        wo = [qr3[h, 1::2, :] for h in hs]
        wrA = np.concatenate([we[0], we[0], we[1], we[1]], 0).T  # [C, 128]
        wrB = np.concatenate([wo[0], wo[0], wo[1], wo[1]], 0).T
        in_maps.append({
            "xT_loc": np.ascontiguousarray(
                xT[:, j * S_LOC:(j + 1) * S_LOC]).astype(BF),
            "wdkvT": wdkvT, "wdqT": wdqT, "bdkv": bdkv, "bdq": bdq,
            "ukT": np.ascontiguousarray(ukT_l.astype(BF)),
            "uqT": np.ascontiguousarray(uqT_l.astype(BF)),
            "uvT": np.ascontiguousarray(uvT_l.astype(BF)),
            "buk": buk2[hs].reshape(2, 128, 1).copy(),
            "buq": buq2[hs].reshape(2, 128, 1).copy(),
            "wrAq": np.ascontiguousarray((wrA * SCALE).astype(BF)),
            "wrBq": np.ascontiguousarray((wrB * SCALE).astype(BF)),
            "wrAk": np.ascontiguousarray(wrA.astype(BF)),
            "wrBk": np.ascontiguousarray(wrB.astype(BF)),
            "c1": c1.astype(np.float16), "c2": c2.astype(np.float16),
            "woT": woT,
        })
    return in_maps


def kernel(**inputs):
    if "nc" not in _CACHE:
        _CACHE["nc"] = _build_program()
    nc = _CACHE["nc"]
    in_maps = _host_prep(inputs)
    res = run_bass_kernel_spmd(nc, in_maps, list(range(NCORES)))
    out = np.concatenate([res.results[j]["out_slice"] for j in range(NCORES)], 0)
    out = out + np.asarray(inputs["out_b"], np.float32)[None, :]
    return out.reshape(B, S, HID)


# revision 22
# speedup vs baseline: 1.0460x; 1.0460x over previous
"""Multi-Head Latent Attention (MLA) Bass kernel for 8 Trainium2 NeuronCores.

Sharding:
  - latent projections (d_kv, d_q): sequence-sharded (BS/8 rows per core),
    per-latent AllGathers replicate the latents (q-latent gathered in fp8).
  - up-projections (u_k, u_v, u_q, qr) + attention: head-sharded, 2 heads/core.
  - context: AllToAll re-shards to sequence; out_proj sequence-parallel.
Structure (v2):
  - dummy 128B AllGather at t=0 absorbs the cross-device rendezvous barrier
    while phase A computes.
  - attention computes scoresT [sk, sq] then ctxT [d, sq] with lhsT = V
    (N=512 moving ops); softmax denominators via DVE pairwise adds +
    ones-vector column-sum matmuls + rank-1 PE broadcast; normalization is a
    single DVE multiply per head-block. No PE transposes.
  - q-side projections (pass-2) are interleaved into the attention block loop
    to fill tensor-engine slack under the scalar-engine exp stream.
  - rope/V biases are zeros by problem spec and are not injected on device;
    out_b is added on host.
"""
import sys
import os

for _p in ("/opt/trn_rl_repo", "/root/.axon_site/_ro/trn_rl_repo"):
    if os.path.isdir(_p) and _p not in sys.path:
        sys.path.insert(0, _p)

import math
import numpy as np
import ml_dtypes

BF = ml_dtypes.bfloat16

import concourse.bacc as bacc
import concourse.mybir as mybir
from concourse import tile
from concourse.bass_utils import run_bass_kernel_spmd

# problem dims (hardcoded)
B, S, H, Dh, Dr, HID, C = 2, 2048, 16, 128, 64, 2048, 512
BS = B * S                      # 4096
NCORES = 8
H_LOC = H // NCORES             # 2
S_LOC = BS // NCORES            # 512
SCALE = 1.0 / math.sqrt(Dh + Dr)

F32 = mybir.dt.float32
F32R = mybir.dt.float32r
F16 = mybir.dt.float16
BF16 = mybir.dt.bfloat16
F8 = mybir.dt.float8e4

_CACHE = {}


def _build_program():
    nc = bacc.Bacc("TRN2", target_bir_lowering=False, debug=False,
                   num_devices=NCORES)

    xT_in = nc.dram_tensor("xT_loc", [HID, S_LOC], BF16, kind="ExternalInput")
    wdkvT = nc.dram_tensor("wdkvT", [HID, C], BF16, kind="ExternalInput")
    wdqT = nc.dram_tensor("wdqT", [HID, C], BF16, kind="ExternalInput")
    bdkv = nc.dram_tensor("bdkv", [4, 128, 1], F32, kind="ExternalInput")
    bdq = nc.dram_tensor("bdq", [4, 128, 1], F32, kind="ExternalInput")
    ukT = nc.dram_tensor("ukT", [C, 256], BF16, kind="ExternalInput")
    uqT = nc.dram_tensor("uqT", [C, 256], BF16, kind="ExternalInput")
    uvT = nc.dram_tensor("uvT", [C, 256], BF16, kind="ExternalInput")
    buk = nc.dram_tensor("buk", [2, 128, 1], F32, kind="ExternalInput")
    buq = nc.dram_tensor("buq", [2, 128, 1], F32, kind="ExternalInput")
    wrAq = nc.dram_tensor("wrAq", [C, 128], BF16, kind="ExternalInput")
    wrBq = nc.dram_tensor("wrBq", [C, 128], BF16, kind="ExternalInput")
    wrAk = nc.dram_tensor("wrAk", [C, 128], BF16, kind="ExternalInput")
    wrBk = nc.dram_tensor("wrBk", [C, 128], BF16, kind="ExternalInput")
    c1_in = nc.dram_tensor("c1", [128, S], F16, kind="ExternalInput")
    c2_in = nc.dram_tensor("c2", [128, S], F16, kind="ExternalInput")

    woT = nc.dram_tensor("woT", [H * Dh, HID], BF16, kind="ExternalInput")
    out_sl = nc.dram_tensor("out_slice", [S_LOC, HID], F32, kind="ExternalOutput")

    groups = [list(range(NCORES))]
    EXP = mybir.ActivationFunctionType.Exp
    IDENT = mybir.ActivationFunctionType.Identity

    with tile.TileContext(nc) as tc:
        with tc.tile_pool(name="dram", bufs=1, space="DRAM") as dram:
            kv_ag_i = dram.tile([4, 128, S_LOC], BF16)
            kv_ag_o = dram.tile([NCORES, 4, 128, S_LOC], BF16,
                                addr_space="Shared", name="kvago")
            ql_ag_i = dram.tile([4, 128, S_LOC], BF16)
            ql_ag_o = dram.tile([NCORES, 4, 128, S_LOC], BF16,
                                addr_space="Shared", name="qlago")
            a2a_i = dram.tile([NCORES, H_LOC * Dh, S_LOC], BF16)
            a2a_o = dram.tile([NCORES, H_LOC * Dh, S_LOC], BF16,
                              name="a2ao")

            with tc.tile_pool(name="const", bufs=1) as const:
                ones_col = const.tile([128, 1], BF16)
                nc.vector.memset(ones_col[:], 1.0)
                ones_f32 = const.tile([1, 128], F32)
                nc.vector.memset(ones_f32[:], 1.0)
                ones_bc = const.tile([1, 128], F32R)
                with nc.allow_low_precision(reason="exact 1.0 constant"):
                    nc.vector.tensor_copy(ones_bc[:], ones_f32[:])

                # ============ Phase A: latents (sequence-sharded) ============
                with tc.tile_pool(name="phA", bufs=1) as phA, \
                     tc.tile_pool(name="phAb", bufs=4) as phAb, \
                     tc.tile_pool(name="psA", bufs=4, space="PSUM") as psA:
                    xts, wkv, wql = [], [], []
                    for ht in range(16):
                        xt = phA.tile([128, S_LOC], BF16, tag=f"xt{ht}",
                                      name=f"xt{ht}")
                        nc.sync.dma_start(out=xt[:],
                                          in_=xT_in[ht * 128:(ht + 1) * 128, :])
                        xts.append(xt)
                        w = phA.tile([128, C], BF16, tag=f"wk{ht}",
                                     name=f"wk{ht}")
                        nc.sync.dma_start(out=w[:],
                                          in_=wdkvT[ht * 128:(ht + 1) * 128, :])
                        wkv.append(w)
                    for ht in range(16):
                        w = phA.tile([128, C], BF16, tag=f"wq{ht}",
                                     name=f"wq{ht}")
                        nc.sync.dma_start(out=w[:],
                                          in_=wdqT[ht * 128:(ht + 1) * 128, :])
                        wql.append(w)
                    for li, (wfull, bdram, agi, ago, ldt) in enumerate(
                            ((wkv, bdkv, kv_ag_i, kv_ag_o, BF16),
                             (wql, bdq, ql_ag_i, ql_ag_o, BF16))):
                        for ct in range(4):
                            ps = psA.tile([128, S_LOC], F32, tag="ps", name="psa")
                            for ht in range(16):
                                nc.tensor.matmul(
                                    ps[:],
                                    wfull[ht][:, ct * 128:(ct + 1) * 128],
                                    xts[ht][:],
                                    start=(ht == 0), stop=(ht == 15))
                            bt = phAb.tile([128, 1], F32, tag="blat", name="blat")
                            nc.sync.dma_start(out=bt[:], in_=bdram[ct])
                            lat = phAb.tile([128, S_LOC], ldt, tag=f"lat{li}",
                                            bufs=3, name=f"lat{li}")
                            nc.vector.tensor_scalar_add(lat[:], ps[:], bt[:])
                            nc.sync.dma_start(out=agi[ct], in_=lat[:])
                        nc.gpsimd.collective_compute(
                            "AllGather", mybir.AluOpType.bypass,
                            replica_groups=groups,
                            ins=[agi.opt()], outs=[ago.opt()])

                # ====== persistent attention operand tiles (phases B-C) ======
                with tc.tile_pool(name="attn", bufs=1) as attn:
                    kcT = [attn.tile([128, BS], BF16, tag=f"kcT{h}",
                                     name=f"kcT{h}") for h in range(2)]
                    qcT = [attn.tile([128, BS], BF16, tag=f"qcT{h}",
                                     name=f"qcT{h}") for h in range(2)]
                    krT = attn.tile([128, BS], BF16, tag="krT", name="krT")
                    qrT = attn.tile([128, BS], BF16, tag="qrT", name="qrT")
                    v_sb = attn.tile([128, 32, 256], BF16, tag="v", name="v_sb")
                    c1s = attn.tile([128, S], F16, tag="c1", name="c1s")
                    nc.sync.dma_start(out=c1s[:], in_=c1_in[:])
                    c2s = attn.tile([128, S], F16, tag="c2", name="c2s")
                    nc.sync.dma_start(out=c2s[:], in_=c2_in[:])

                    # ========= pass 1: kv-dependent (k_c, rope-k, V) =========
                    with tc.tile_pool(name="phB", bufs=1) as phB, \
                         tc.tile_pool(name="phBt", bufs=2) as phBt, \
                         tc.tile_pool(name="psB", bufs=2, space="PSUM") as psB:
                        upw = {}
                        for nm, t, w_ in (("uk", ukT, 256), ("uv", uvT, 256),
                                          ("wrAk", wrAk, 128),
                                          ("wrBk", wrBk, 128)):
                            tl = []
                            for ct in range(4):
                                wt = phB.tile([128, w_], BF16, tag=f"{nm}{ct}",
                                              name=f"{nm}{ct}")
                                nc.sync.dma_start(
                                    out=wt[:],
                                    in_=t[ct * 128:(ct + 1) * 128, :])
                                tl.append(wt)
                            upw[nm] = tl
                        buk_t = []
                        for h in range(2):
                            bt_ = phB.tile([128, 1], F32, tag=f"buk{h}",
                                           name=f"buk{h}")
                            nc.sync.dma_start(out=bt_[:], in_=buk[h])
                            buk_t.append(bt_)

                        # keep the PE clock un-throttled across the AG-kv
                        # wait: write-only filler matmuls on resident weights
                        # (results never read).
                        psj = psB.tile([128, 256], F32, tag="junk",
                                       bufs=1, name="psj")
                        for _ in range(300):
                            nc.tensor.matmul(psj[:],
                                             upw["uk"][0][:, 0:128],
                                             upw["uv"][0][:],
                                             start=True, stop=True)

                        for j2 in range(8):
                            sl = slice(j2 * 512, (j2 + 1) * 512)
                            pos = slice((j2 % 4) * 512, (j2 % 4) * 512 + 512)
                            kv_sb = []
                            for ct in range(4):
                                kt_ = phBt.tile([128, 512], BF16,
                                                tag=f"kv{j2}_{ct}", bufs=1,
                                                name=f"kv{j2}_{ct}")
                                nc.sync.dma_start(out=kt_[:],
                                                  in_=kv_ag_o[j2, ct])
                                kv_sb.append(kt_)
                            for h in range(2):
                                hc = slice(h * 128, (h + 1) * 128)
                                ps = psB.tile([128, 512], F32, tag="psKC",
                                              name="pskc")
                                for ct in range(4):
                                    nc.tensor.matmul(ps[:],
                                                     upw["uk"][ct][:, hc],
                                                     kv_sb[ct][:],
                                                     start=(ct == 0),
                                                     stop=(ct == 3))
                                nc.scalar.activation(
                                    kcT[h][:, sl], ps[:], IDENT,
                                    bias=buk_t[h][:])
                            psa_ = psB.tile([128, 512], F32, tag="psRA",
                                            bufs=1, name="psra")
                            psb_ = psB.tile([128, 512], F32, tag="psRB",
                                            bufs=1, name="psrb")
                            for ct in range(4):
                                nc.tensor.matmul(psa_[:], upw["wrAk"][ct][:],
                                                 kv_sb[ct][:],
                                                 start=(ct == 0), stop=(ct == 3))
                            for ct in range(4):
                                nc.tensor.matmul(psb_[:], upw["wrBk"][ct][:],
                                                 kv_sb[ct][:],
                                                 start=(ct == 0), stop=(ct == 3))
                            t1 = phBt.tile([128, 512], F32, tag="t1", bufs=2,
                                           name="t1")
                            nc.vector.tensor_mul(t1[:], psa_[:], c1s[:, pos])
                            t2 = phBt.tile([128, 512], F32, tag="t2", bufs=2,
                                           name="t2")
                            nc.vector.tensor_mul(t2[:], psb_[:], c2s[:, pos])
                            nc.vector.tensor_add(krT[:, sl], t1[:], t2[:])
                            for ss in range(4):
                                psv_ = psB.tile([128, 256], F32, tag="psV",
                                                name="psv")
                                ssl = slice(ss * 128, (ss + 1) * 128)
                                for ct in range(4):
                                    nc.tensor.matmul(psv_[:],
                                                     kv_sb[ct][:, ssl],
                                                     upw["uv"][ct][:],
                                                     start=(ct == 0),
                                                     stop=(ct == 3))
                                st = j2 * 4 + ss
                                nc.vector.tensor_copy(v_sb[:, st, :], psv_[:])

                    # ====== persistent q-side weights (used inside C) ======
                    uqw, wrq = [], {}
                    for ct in range(4):
                        wt = attn.tile([128, 256], BF16, tag=f"uq{ct}",
                                       name=f"uq{ct}")
                        nc.sync.dma_start(out=wt[:],
                                          in_=uqT[ct * 128:(ct + 1) * 128, :])
                        uqw.append(wt)
                    for nm, t in (("wrAq", wrAq), ("wrBq", wrBq)):
                        tl = []
                        for ct in range(4):
                            wt = attn.tile([128, 128], BF16, tag=f"{nm}{ct}",
                                           name=f"{nm}{ct}")
                            nc.sync.dma_start(
                                out=wt[:], in_=t[ct * 128:(ct + 1) * 128, :])
                            tl.append(wt)
                        wrq[nm] = tl
                    buq_t = []
                    for h in range(2):
                        bt_ = attn.tile([128, 1], F32, tag=f"buq{h}",
                                        name=f"buq{h}")
                        nc.sync.dma_start(out=bt_[:], in_=buq[h])
                        buq_t.append(bt_)

                    with tc.tile_pool(name="phDw0", bufs=1) as phDw0:
                        # ================= Phase C: attention =================
                        with tc.tile_pool(name="phC", bufs=1) as phC, \
                             tc.tile_pool(name="psC", bufs=1,
                                          space="PSUM") as psC:
                            def pass2(g):
                                """q-side projections for seq block g."""
                                if g >= 8:
                                    return
                                sl = slice(g * 512, (g + 1) * 512)
                                pos = slice((g % 4) * 512, (g % 4) * 512 + 512)
                                ql_sb = []
                                for ct in range(4):
                                    qs = phC.tile([128, 512], BF16,
                                                  tag=f"qs{ct}", bufs=2,
                                                  name=f"qs{ct}")
                                    nc.sync.dma_start(out=qs[:],
                                                      in_=ql_ag_o[g, ct])
                                    ql_sb.append(qs)
                                psq = psC.tile([128, 2, 512], F32, tag="ps2",
                                               bufs=2, name="psq")
                                for h in range(2):
                                    hc = slice(h * 128, (h + 1) * 128)
                                    for ct in range(4):
                                        nc.tensor.matmul(psq[:, h, :],
                                                         uqw[ct][:, hc],
                                                         ql_sb[ct][:],
                                                         start=(ct == 0),
                                                         stop=(ct == 3))
                                for h in range(2):
                                    nc.vector.tensor_scalar_add(
                                        qcT[h][:, sl], psq[:, h, :],
                                        buq_t[h][:])
                                psr = psC.tile([128, 2, 512], F32, tag="ps2",
                                               bufs=2, name="psr")
                                for ct in range(4):
                                    nc.tensor.matmul(psr[:, 0, :],
                                                     wrq["wrAq"][ct][:],
                                                     ql_sb[ct][:],
                                                     start=(ct == 0),
                                                     stop=(ct == 3))
                                for ct in range(4):
                                    nc.tensor.matmul(psr[:, 1, :],
                                                     wrq["wrBq"][ct][:],
                                                     ql_sb[ct][:],
                                                     start=(ct == 0),
                                                     stop=(ct == 3))
                                t1 = phC.tile([128, 512], F32, tag="t1",
                                              bufs=1, name="t1c")
                                nc.vector.tensor_mul(t1[:], psr[:, 0, :],
                                                     c1s[:, pos])
                                t2 = phC.tile([128, 512], F32, tag="t2",
                                              bufs=1, name="t2c")
                                nc.vector.tensor_mul(t2[:], psr[:, 1, :],
                                                     c2s[:, pos])
                                nc.vector.tensor_add(qrT[:, sl], t1[:], t2[:])

                            def denom_dve(probs_p):
                                """bf16 pairwise-add tree over sk tiles."""
                                p8 = phC.tile([128, 2, 8, 512], BF16,
                                              tag="p8", bufs=1, name="p8")
                                for h in range(2):
                                    nc.vector.tensor_add(
                                        p8[:, h, :, :],
                                        probs_p[:, 0:8, h, :],
                                        probs_p[:, 8:16, h, :])
                                p4 = phC.tile([128, 2, 4, 512], BF16,
                                              tag="p4", bufs=1, name="p4")
                                for h in range(2):
                                    nc.vector.tensor_add(
                                        p4[:, h, :, :],
                                        p8[:, h, 0:4, :],
                                        p8[:, h, 4:8, :])
                                return p4

                            def denom_pe(p4, pe_bcast=False):
                                """column-sum + reciprocal + broadcast."""
                                dsum = psC.tile([128, 2, 512], F32,
                                                tag="dsum", bufs=1,
                                                name="dsum")
                                for h in range(2):
                                    for i in range(4):
                                        nc.tensor.matmul(
                                            dsum[0:1, h, :], ones_col[:],
                                            p4[:, h, i, :],
                                            start=(i == 0), stop=(i == 3))
                                rec = phC.tile([1, 2, 512], F32R, tag="rec",
                                               bufs=1, name="rec")
                                with nc.allow_low_precision(
                                        reason="f32r keeps ~13 mantissa "
                                        "bits; ample for softmax "
                                        "denominators"):
                                    for h in range(2):
                                        nc.vector.reciprocal(rec[0:1, h, :],
                                                             dsum[0:1, h, :])
                                recb = phC.tile([128, 2, 512], F32R,
                                                tag="recb", bufs=1,
                                                name="recb")
                                if pe_bcast:
                                    for h in range(2):
                                        nc.tensor.matmul(dsum[:, h, :],
                                                         ones_bc[:],
                                                         rec[0:1, h, :],
                                                         start=True,
                                                         stop=True)
                                    nc.vector.tensor_copy(recb[:], dsum[:])
                                else:
                                    nc.gpsimd.partition_broadcast(
                                        recb[:], rec[0:1, :, :])
                                return recb

                            def finish_stg(prev, psvs, recb):
                                pg, pb, probs_p = prev
                                for h in range(2):
                                    stg = phC.tile([128, 512], BF16,
                                                   tag=f"stg{h}", bufs=1,
                                                   name=f"stg{h}")
                                    nc.vector.tensor_mul(stg[:], psvs[h][:],
                                                         recb[:, h, :])
                                    nc.sync.dma_start(
                                        out=a2a_i[pg,
                                                  h * 128:(h + 1) * 128, :],
                                        in_=stg[:])

                            pass2(0)
                            pass2(1)
                            # software pipeline: block g's scores+exp stream
                            # carries block g-1's PV + denominator matmuls so
                            # the tensor engine never drains behind the
                            # scalar-engine exp chain. Column-sums for g-1 are
                            # issued right after the second score group so the
                            # reciprocal/broadcast chain retires mid-iteration.
                            prev = None
                            for b in range(2):
                                for sqb in range(4):
                                    g = b * 4 + sqb
                                    qsl = slice(b * S + sqb * 512,
                                                b * S + sqb * 512 + 512)
                                    probs = phC.tile([128, 16, 2, 512], BF16,
                                                     tag="probs", bufs=2,
                                                     name="probs")
                                    if prev is not None:
                                        p4p = denom_dve(prev[2])
                                        psvs = [psC.tile([128, 512], F32,
                                                         tag=f"psv{h}",
                                                         bufs=1,
                                                         name=f"psv{h}")
                                                for h in range(2)]
                                    recb = None
                                    for skt in range(16):
                                        ps2 = psC.tile([128, 2, 512], F32,
                                                       tag="ps2", bufs=2,
                                                       name="ps2")
                                        ksl = slice(b * S + skt * 128,
                                                    b * S + skt * 128 + 128)
                                        for h in range(2):
                                            nc.tensor.matmul(
                                                ps2[:, h, :],
                                                kcT[h][:, ksl],
                                                qcT[h][:, qsl],
                                                start=True, stop=False)
                                        for h in range(2):
                                            hr = slice(h * 64, h * 64 + 64)
                                            nc.tensor.matmul(
                                                ps2[:, h, :],
                                                krT[hr, ksl],
                                                qrT[hr, qsl],
                                                start=False, stop=True)
                                        nc.scalar.activation(
                                            probs[:, skt, :, :], ps2[:], EXP)
                                        if prev is not None:
                                            pg, pb, probs_p = prev
                                            for h in range(2):
                                                nc.tensor.matmul(
                                                    psvs[h][:],
                                                    v_sb[:, pb * 16 + skt,
                                                         h * 128:
                                                         (h + 1) * 128],
                                                    probs_p[:, skt, h, :],
                                                    start=(skt == 0),
                                                    stop=(skt == 15))
                                            if skt == 5:
                                                recb = denom_pe(p4p)
                                    if prev is not None:
                                        finish_stg(prev, psvs, recb)
                                    pass2(g + 2)
                                    prev = (g, b, probs)
                            # drain the last block
                            pg, pb, probs_p = prev
                            p4p = denom_dve(probs_p)
                            psvs = [psC.tile([128, 512], F32, tag=f"psv{h}",
                                             bufs=1, name=f"psv{h}")
                                    for h in range(2)]
                            for skt in range(16):
                                for h in range(2):
                                    nc.tensor.matmul(
                                        psvs[h][:],
                                        v_sb[:, pb * 16 + skt,
                                             h * 128:(h + 1) * 128],
                                        probs_p[:, skt, h, :],
                                        start=(skt == 0), stop=(skt == 15))
                            finish_stg(prev, psvs,
                                       denom_pe(p4p, pe_bcast=True))
                            nc.gpsimd.collective_compute(
                                "AllToAll", mybir.AluOpType.bypass,
                                replica_groups=groups,
                                ins=[a2a_i.opt()], outs=[a2a_o.opt()])

                        # ============== Phase D: out projection ==============
                        with tc.tile_pool(name="phD", bufs=1) as phD, \
                             tc.tile_pool(name="phDw", bufs=2) as phDw, \
                             tc.tile_pool(name="phDo", bufs=3) as phDo, \
                             tc.tile_pool(name="psD", bufs=2,
                                          space="PSUM") as psD:
                            wos0 = []
                            for dht in range(16):
                                wo = phDw0.tile([128, 512], BF16,
                                                tag=f"wo0{dht}",
                                                name=f"wo0{dht}")
                                nc.scalar.dma_start(
                                    out=wo[:],
                                    in_=woT[dht * 128:(dht + 1) * 128,
                                            0:512])
                                wos0.append(wo)
                            psj2 = psD.tile([128, 512], F32, tag="junk",
                                            bufs=1, name="psj2")
                            for _ in range(180):
                                nc.tensor.matmul(psj2[:],
                                                 wos0[0][:, 0:128],
                                                 wos0[1][:],
                                                 start=True, stop=True)
                            csl = []
                            for dht in range(16):
                                cf = phD.tile([128, S_LOC], BF16,
                                              tag=f"cf{dht}", name=f"cf{dht}")
                                nc.sync.dma_start(
                                    out=cf[:],
                                    in_=a2a_o[dht // 2,
                                              (dht % 2) * 128:
                                              (dht % 2) * 128 + 128, :])
                                csl.append(cf)
                            for ot in range(4):
                                osl = slice(ot * 512, (ot + 1) * 512)
                                if ot == 0:
                                    wos = wos0
                                else:
                                    wos = []
                                    for dht in range(16):
                                        wo = phDw.tile([128, 512], BF16,
                                                       tag=f"wo{dht}",
                                                       name=f"wo{dht}")
                                        nc.sync.dma_start(
                                            out=wo[:],
                                            in_=woT[dht * 128:(dht + 1) * 128,
                                                    osl])
                                        wos.append(wo)
                                for ssub in range(4):
                                    pso = psD.tile([128, 512], F32, tag="psO",
                                                   name="pso")
                                    ssl = slice(ssub * 128, (ssub + 1) * 128)
                                    for dht in range(16):
                                        nc.tensor.matmul(pso[:],
                                                         csl[dht][:, ssl],
                                                         wos[dht][:],
                                                         start=(dht == 0),
                                                         stop=(dht == 15))
                                    osb = phDo.tile([128, 512], F32,
                                                    tag="osb", name="osb")
                                    nc.vector.tensor_copy(osb[:], pso[:])
                                    nc.sync.dma_start(out=out_sl[ssl, osl],
                                                      in_=osb[:])

    nc.compile()
    return nc


def _host_prep(inputs):
    """Build per-core input maps from the full problem inputs."""
    x = np.asarray(inputs["x"], np.float32)
    xT = np.ascontiguousarray(x.reshape(BS, HID).T)            # [HID, BS]
    wdkvT = np.ascontiguousarray(np.asarray(inputs["d_kv_w"], np.float32).T.astype(BF))
    wdqT = np.ascontiguousarray(np.asarray(inputs["d_q_w"], np.float32).T.astype(BF))
    bdkv = np.asarray(inputs["d_kv_b"], np.float32).reshape(4, 128, 1)
    bdq = np.asarray(inputs["d_q_b"], np.float32).reshape(4, 128, 1)

    uk3 = np.asarray(inputs["u_k_w"], np.float32).reshape(H, Dh, C)
    uq3 = np.asarray(inputs["u_q_w"], np.float32).reshape(H, Dh, C) * SCALE
    uv3 = np.asarray(inputs["u_v_w"], np.float32).reshape(H, Dh, C)
    buk2 = np.asarray(inputs["u_k_b"], np.float32).reshape(H, Dh)
    buq2 = np.asarray(inputs["u_q_b"], np.float32).reshape(H, Dh) * SCALE
    qr3 = np.asarray(inputs["qr_w"], np.float32).reshape(H, Dr, C)

    # rope tables (positions 0..S-1)
    i32 = np.arange(32, dtype=np.float32)
    inv_freq = (10000.0 ** (-(2.0 * i32) / Dr)).astype(np.float32)  # [32]
    pos = np.arange(S, dtype=np.float32)
    ang = pos[None, :] * inv_freq[:, None]                     # [32, S]
    cos, sin = np.cos(ang), np.sin(ang)
    c1 = np.concatenate([cos, sin, cos, sin], 0).astype(np.float32)
    c2 = np.concatenate([-sin, cos, -sin, cos], 0).astype(np.float32)

    woT = np.ascontiguousarray(np.asarray(inputs["out_w"], np.float32).T.astype(BF))

    in_maps = []
    for j in range(NCORES):
        hs = [2 * j, 2 * j + 1]
        ukT_l = uk3[hs].transpose(2, 0, 1).reshape(C, 256)
        uqT_l = uq3[hs].transpose(2, 0, 1).reshape(C, 256)
        uvT_l = uv3[hs].transpose(2, 0, 1).reshape(C, 256)
        we = [qr3[h, 0::2, :] for h in hs]    # [32, C] each
        wo = [qr3[h, 1::2, :] for h in hs]
        wrA = np.concatenate([we[0], we[0], we[1], we[1]], 0).T  # [C, 128]
        wrB = np.concatenate([wo[0], wo[0], wo[1], wo[1]], 0).T
        in_maps.append({
            "xT_loc": np.ascontiguousarray(
                xT[:, j * S_LOC:(j + 1) * S_LOC]).astype(BF),
            "wdkvT": wdkvT, "wdqT": wdqT, "bdkv": bdkv, "bdq": bdq,
            "ukT": np.ascontiguousarray(ukT_l.astype(BF)),
            "uqT": np.ascontiguousarray(uqT_l.astype(BF)),
            "uvT": np.ascontiguousarray(uvT_l.astype(BF)),
            "buk": buk2[hs].reshape(2, 128, 1).copy(),
            "buq": buq2[hs].reshape(2, 128, 1).copy(),
            "wrAq": np.ascontiguousarray((wrA * SCALE).astype(BF)),
            "wrBq": np.ascontiguousarray((wrB * SCALE).astype(BF)),
            "wrAk": np.ascontiguousarray(wrA.astype(BF)),
            "wrBk": np.ascontiguousarray(wrB.astype(BF)),
            "c1": c1.astype(np.float16), "c2": c2.astype(np.float16),
            "woT": woT,
        })
    return in_maps


def kernel(**inputs):
    if "nc" not in _CACHE:
        _CACHE["nc"] = _build_program()
    nc = _CACHE["nc"]
    in_maps = _host_prep(inputs)
    res = run_bass_kernel_spmd(nc, in_maps, list(range(NCORES)))
    out = np.concatenate([res.results[j]["out_slice"] for j in range(NCORES)], 0)
    out = out + np.asarray(inputs["out_b"], np.float32)[None, :]
    return out.reshape(B, S, HID)


# revision 32
# speedup vs baseline: 1.0558x; 1.0093x over previous
"""Multi-Head Latent Attention (MLA) Bass kernel for 8 Trainium2 NeuronCores.

Sharding:
  - latent projections (d_kv, d_q): sequence-sharded (BS/8 rows per core),
    per-latent bf16 AllGathers (Shared outputs) replicate the latents.
  - up-projections (u_k, u_v, u_q, qr) + attention: head-sharded, 2 heads/core.
  - context: AllToAll re-shards to sequence; out_proj sequence-parallel.
Structure (v3):
  - attention computes scoresT [sk, sq] then ctxT [d, sq] with lhsT = V
    (N=512 moving operands, no PE transposes). Softmax denominators: bf16
    pairwise-add tree on DVE, ones-vector column-sum matmuls, fp32
    Newton-Raphson reciprocal (multiply-path, ~3x the iterative-divide
    rate), gpsimd partition_broadcast (PE rank-1 broadcast on the drain
    block); normalization is one DVE multiply per head-block.
  - software pipeline: block g's score/exp stream carries block g-1's PV and
    denominator matmuls, so the tensor engine never drains behind the
    scalar-engine exp chain (keeps the HAM clock-gate warm).
  - q-side projections (pass-2) are interleaved into the attention block loop
    to fill tensor-engine slack; their DMA loads issue on the sync queue
    ahead of use.
  - write-only filler matmuls keep the PE busy across the two exposed
    collective waits (first AllGather, AllToAll) to avoid HAM re-throttle.
  - rope/V biases are zeros by problem spec and are not injected on device;
    out_b is added on host.
"""
import sys
import os

for _p in ("/opt/trn_rl_repo", "/root/.axon_site/_ro/trn_rl_repo"):
    if os.path.isdir(_p) and _p not in sys.path:
        sys.path.insert(0, _p)

import math
import numpy as np
import ml_dtypes

BF = ml_dtypes.bfloat16

import concourse.bacc as bacc
import concourse.mybir as mybir
from concourse import tile
from concourse.bass_utils import run_bass_kernel_spmd

# problem dims (hardcoded)
B, S, H, Dh, Dr, HID, C = 2, 2048, 16, 128, 64, 2048, 512
BS = B * S                      # 4096
NCORES = 8
H_LOC = H // NCORES             # 2
S_LOC = BS // NCORES            # 512
SCALE = 1.0 / math.sqrt(Dh + Dr)

F32 = mybir.dt.float32
F32R = mybir.dt.float32r
F16 = mybir.dt.float16
BF16 = mybir.dt.bfloat16
F8 = mybir.dt.float8e4

_CACHE = {}


def _build_program():
    nc = bacc.Bacc("TRN2", target_bir_lowering=False, debug=False,
                   num_devices=NCORES)

    xT_in = nc.dram_tensor("xT_loc", [HID, S_LOC], BF16, kind="ExternalInput")
    wdkvT = nc.dram_tensor("wdkvT", [HID, C], BF16, kind="ExternalInput")
    wdqT = nc.dram_tensor("wdqT", [HID, C], BF16, kind="ExternalInput")
    bdkv = nc.dram_tensor("bdkv", [4, 128, 1], F32, kind="ExternalInput")
    bdq = nc.dram_tensor("bdq", [4, 128, 1], F32, kind="ExternalInput")
    ukT = nc.dram_tensor("ukT", [C, 256], BF16, kind="ExternalInput")
    uqT = nc.dram_tensor("uqT", [C, 256], BF16, kind="ExternalInput")
    uvT = nc.dram_tensor("uvT", [C, 256], BF16, kind="ExternalInput")
    buk = nc.dram_tensor("buk", [2, 128, 1], F32, kind="ExternalInput")
    buq = nc.dram_tensor("buq", [2, 128, 1], F32, kind="ExternalInput")
    wrAq = nc.dram_tensor("wrAq", [C, 128], BF16, kind="ExternalInput")
    wrBq = nc.dram_tensor("wrBq", [C, 128], BF16, kind="ExternalInput")
    wrAk = nc.dram_tensor("wrAk", [C, 128], BF16, kind="ExternalInput")
    wrBk = nc.dram_tensor("wrBk", [C, 128], BF16, kind="ExternalInput")
    c1_in = nc.dram_tensor("c1", [128, S], F16, kind="ExternalInput")
    c2_in = nc.dram_tensor("c2", [128, S], F16, kind="ExternalInput")

    woT = nc.dram_tensor("woT", [H * Dh, HID], BF16, kind="ExternalInput")
    out_sl = nc.dram_tensor("out_slice", [S_LOC, HID], F32, kind="ExternalOutput")

    groups = [list(range(NCORES))]
    EXP = mybir.ActivationFunctionType.Exp
    IDENT = mybir.ActivationFunctionType.Identity

    with tile.TileContext(nc) as tc:
        with tc.tile_pool(name="dram", bufs=1, space="DRAM") as dram:
            kv_ag_i = dram.tile([4, 128, S_LOC], BF16)
            kv_ag_o = dram.tile([NCORES, 4, 128, S_LOC], BF16,
                                addr_space="Shared", name="kvago")
            ql_ag_i = dram.tile([4, 128, S_LOC], BF16)
            ql_ag_o = dram.tile([NCORES, 4, 128, S_LOC], BF16,
                                addr_space="Shared", name="qlago")
            a2a_i = dram.tile([NCORES, H_LOC * Dh, S_LOC], BF16)
            a2a_o = dram.tile([NCORES, H_LOC * Dh, S_LOC], BF16,
                              name="a2ao")

            with tc.tile_pool(name="const", bufs=1) as const:
                ones_col = const.tile([128, 1], BF16)
                nc.vector.memset(ones_col[:], 1.0)
                ones_f32 = const.tile([1, 128], F32)
                nc.vector.memset(ones_f32[:], 1.0)

                # ============ Phase A: latents (sequence-sharded) ============
                with tc.tile_pool(name="phA", bufs=1) as phA, \
                     tc.tile_pool(name="phAb", bufs=4) as phAb, \
                     tc.tile_pool(name="psA", bufs=4, space="PSUM") as psA:
                    xts, wkv, wql = [], [], []
                    for ht in range(16):
                        xt = phA.tile([128, S_LOC], BF16, tag=f"xt{ht}",
                                      name=f"xt{ht}")
                        nc.sync.dma_start(out=xt[:],
                                          in_=xT_in[ht * 128:(ht + 1) * 128, :])
                        xts.append(xt)
                        w = phA.tile([128, C], BF16, tag=f"wk{ht}",
                                     name=f"wk{ht}")
                        nc.sync.dma_start(out=w[:],
                                          in_=wdkvT[ht * 128:(ht + 1) * 128, :])
                        wkv.append(w)
                    for ht in range(16):
                        w = phA.tile([128, C], BF16, tag=f"wq{ht}",
                                     name=f"wq{ht}")
                        nc.sync.dma_start(out=w[:],
                                          in_=wdqT[ht * 128:(ht + 1) * 128, :])
                        wql.append(w)
                    for li, (wfull, bdram, agi, ago, ldt) in enumerate(
                            ((wkv, bdkv, kv_ag_i, kv_ag_o, BF16),
                             (wql, bdq, ql_ag_i, ql_ag_o, BF16))):
                        for ct in range(4):
                            ps = psA.tile([128, S_LOC], F32, tag="ps", name="psa")
                            for ht in range(16):
                                nc.tensor.matmul(
                                    ps[:],
                                    wfull[ht][:, ct * 128:(ct + 1) * 128],
                                    xts[ht][:],
                                    start=(ht == 0), stop=(ht == 15))
                            bt = phAb.tile([128, 1], F32, tag="blat", name="blat")
                            nc.sync.dma_start(out=bt[:], in_=bdram[ct])
                            lat = phAb.tile([128, S_LOC], ldt, tag=f"lat{li}",
                                            bufs=3, name=f"lat{li}")
                            nc.vector.tensor_scalar_add(lat[:], ps[:], bt[:])
                            nc.sync.dma_start(out=agi[ct], in_=lat[:])
                        nc.gpsimd.collective_compute(
                            "AllGather", mybir.AluOpType.bypass,
                            replica_groups=groups,
                            ins=[agi.opt()], outs=[ago.opt()])

                # preload the gpsimd extended-instruction library now (both
                # AllGather triggers above already ran on the default lib) so
                # the first real partition_broadcast doesn't pay the ~9us
                # LOAD_LIB swap mid-attention.
                pbin = const.tile([1, 64], F32)
                nc.vector.memset(pbin[:], 1.0)
                pbout = const.tile([128, 64], F32)
                nc.gpsimd.partition_broadcast(pbout[:], pbin[:])

                # ====== persistent attention operand tiles (phases B-C) ======
                with tc.tile_pool(name="attn", bufs=1) as attn:
                    kcT = [attn.tile([128, BS], BF16, tag=f"kcT{h}",
                                     name=f"kcT{h}") for h in range(2)]
                    qcT = [attn.tile([128, BS], BF16, tag=f"qcT{h}",
                                     name=f"qcT{h}") for h in range(2)]
                    krT = attn.tile([128, BS], BF16, tag="krT", name="krT")
                    qrT = attn.tile([128, BS], BF16, tag="qrT", name="qrT")
                    v_sb = attn.tile([128, 32, 256], BF16, tag="v", name="v_sb")
                    c1s = attn.tile([128, S], F16, tag="c1", name="c1s")
                    nc.sync.dma_start(out=c1s[:], in_=c1_in[:])
                    c2s = attn.tile([128, S], F16, tag="c2", name="c2s")
                    nc.sync.dma_start(out=c2s[:], in_=c2_in[:])

                    # ========= pass 1: kv-dependent (k_c, rope-k, V) =========
                    with tc.tile_pool(name="phB", bufs=1) as phB, \
                         tc.tile_pool(name="phBt", bufs=2) as phBt, \
                         tc.tile_pool(name="psB", bufs=2, space="PSUM") as psB:
                        upw = {}
                        for nm, t, w_ in (("uk", ukT, 256), ("uv", uvT, 256),
                                          ("wrAk", wrAk, 128),
                                          ("wrBk", wrBk, 128)):
                            tl = []
                            for ct in range(4):
                                wt = phB.tile([128, w_], BF16, tag=f"{nm}{ct}",
                                              name=f"{nm}{ct}")
                                nc.sync.dma_start(
                                    out=wt[:],
                                    in_=t[ct * 128:(ct + 1) * 128, :])
                                tl.append(wt)
                            upw[nm] = tl
                        buk_t = []
                        for h in range(2):
                            bt_ = phB.tile([128, 1], F32, tag=f"buk{h}",
                                           name=f"buk{h}")
                            nc.sync.dma_start(out=bt_[:], in_=buk[h])
                            buk_t.append(bt_)

                        # keep the PE clock un-throttled across the AG-kv
                        # wait: write-only filler matmuls on resident weights
                        # (results never read).
                        psj = psB.tile([128, 256], F32, tag="junk",
                                       bufs=1, name="psj")
                        for _ in range(400):
                            nc.tensor.matmul(psj[:],
                                             upw["uk"][0][:, 0:128],
                                             upw["uv"][0][:],
                                             start=True, stop=True)

                        for j2 in range(8):
                            sl = slice(j2 * 512, (j2 + 1) * 512)
                            pos = slice((j2 % 4) * 512, (j2 % 4) * 512 + 512)
                            kv_sb = []
                            for ct in range(4):
                                kt_ = phBt.tile([128, 512], BF16,
                                                tag=f"kv{j2}_{ct}", bufs=1,
                                                name=f"kv{j2}_{ct}")
                                eng = nc.sync if ct % 2 == 0 else nc.scalar
                                eng.dma_start(out=kt_[:],
                                              in_=kv_ag_o[j2, ct])
                                kv_sb.append(kt_)
                            for h in range(2):
                                hc = slice(h * 128, (h + 1) * 128)
                                ps = psB.tile([128, 512], F32, tag="psKC",
                                              name="pskc")
                                for ct in range(4):
                                    nc.tensor.matmul(ps[:],
                                                     upw["uk"][ct][:, hc],
                                                     kv_sb[ct][:],
                                                     start=(ct == 0),
                                                     stop=(ct == 3))
                                nc.scalar.activation(
                                    kcT[h][:, sl], ps[:], IDENT,
                                    bias=buk_t[h][:])
                            psa_ = psB.tile([128, 512], F32, tag="psRA",
                                            bufs=1, name="psra")
                            psb_ = psB.tile([128, 512], F32, tag="psRB",
                                            bufs=1, name="psrb")
                            for ct in range(4):
                                nc.tensor.matmul(psa_[:], upw["wrAk"][ct][:],
                                                 kv_sb[ct][:],
                                                 start=(ct == 0), stop=(ct == 3))
                            for ct in range(4):
                                nc.tensor.matmul(psb_[:], upw["wrBk"][ct][:],
                                                 kv_sb[ct][:],
                                                 start=(ct == 0), stop=(ct == 3))
                            t1 = phBt.tile([128, 512], F32, tag="t1", bufs=2,
                                           name="t1")
                            nc.vector.tensor_mul(t1[:], psa_[:], c1s[:, pos])
                            t2 = phBt.tile([128, 512], F32, tag="t2", bufs=2,
                                           name="t2")
                            nc.vector.tensor_mul(t2[:], psb_[:], c2s[:, pos])
                            nc.vector.tensor_add(krT[:, sl], t1[:], t2[:])
                            for ss in range(4):
                                psv_ = psB.tile([128, 256], F32, tag="psV",
                                                name="psv")
                                ssl = slice(ss * 128, (ss + 1) * 128)
                                for ct in range(4):
                                    nc.tensor.matmul(psv_[:],
                                                     kv_sb[ct][:, ssl],
                                                     upw["uv"][ct][:],
                                                     start=(ct == 0),
                                                     stop=(ct == 3))
                                st = j2 * 4 + ss
                                nc.vector.tensor_copy(v_sb[:, st, :], psv_[:])

                    # ====== persistent q-side weights (used inside C) ======
                    uqw, wrq = [], {}
                    for ct in range(4):
                        wt = attn.tile([128, 256], BF16, tag=f"uq{ct}",
                                       name=f"uq{ct}")
                        nc.sync.dma_start(out=wt[:],
                                          in_=uqT[ct * 128:(ct + 1) * 128, :])
                        uqw.append(wt)
                    for nm, t in (("wrAq", wrAq), ("wrBq", wrBq)):
                        tl = []
                        for ct in range(4):
                            wt = attn.tile([128, 128], BF16, tag=f"{nm}{ct}",
                                           name=f"{nm}{ct}")
                            nc.sync.dma_start(
                                out=wt[:], in_=t[ct * 128:(ct + 1) * 128, :])
                            tl.append(wt)
                        wrq[nm] = tl
                    buq_t = []
                    for h in range(2):
                        bt_ = attn.tile([128, 1], F32, tag=f"buq{h}",
                                        name=f"buq{h}")
                        nc.sync.dma_start(out=bt_[:], in_=buq[h])
                        buq_t.append(bt_)

                    with tc.tile_pool(name="phDw0", bufs=1) as phDw0:
                        # ================= Phase C: attention =================
                        with tc.tile_pool(name="phC", bufs=1) as phC, \
                             tc.tile_pool(name="psC", bufs=1,
                                          space="PSUM") as psC:
                            def pass2(g):
                                """q-side projections for seq block g."""
                                if g >= 8:
                                    return
                                sl = slice(g * 512, (g + 1) * 512)
                                pos = slice((g % 4) * 512, (g % 4) * 512 + 512)
                                ql_sb = []
                                for ct in range(4):
                                    qs = phC.tile([128, 512], BF16,
                                                  tag=f"qs{ct}", bufs=2,
                                                  name=f"qs{ct}")
                                    nc.sync.dma_start(out=qs[:],
                                                      in_=ql_ag_o[g, ct])
                                    ql_sb.append(qs)
                                psq = psC.tile([128, 2, 512], F32, tag="ps2",
                                               bufs=2, name="psq")
                                for h in range(2):
                                    hc = slice(h * 128, (h + 1) * 128)
                                    for ct in range(4):
                                        nc.tensor.matmul(psq[:, h, :],
                                                         uqw[ct][:, hc],
                                                         ql_sb[ct][:],
                                                         start=(ct == 0),
                                                         stop=(ct == 3))
                                for h in range(2):
                                    nc.vector.tensor_scalar_add(
                                        qcT[h][:, sl], psq[:, h, :],
                                        buq_t[h][:])
                                psr = psC.tile([128, 2, 512], F32, tag="ps2",
                                               bufs=2, name="psr")
                                for ct in range(4):
                                    nc.tensor.matmul(psr[:, 0, :],
                                                     wrq["wrAq"][ct][:],
                                                     ql_sb[ct][:],
                                                     start=(ct == 0),
                                                     stop=(ct == 3))
                                for ct in range(4):
                                    nc.tensor.matmul(psr[:, 1, :],
                                                     wrq["wrBq"][ct][:],
                                                     ql_sb[ct][:],
                                                     start=(ct == 0),
                                                     stop=(ct == 3))
                                t1 = phC.tile([128, 512], F32, tag="t1",
                                              bufs=1, name="t1c")
                                nc.vector.tensor_mul(t1[:], psr[:, 0, :],
                                                     c1s[:, pos])
                                t2 = phC.tile([128, 512], F32, tag="t2",
                                              bufs=1, name="t2c")
                                nc.vector.tensor_mul(t2[:], psr[:, 1, :],
                                                     c2s[:, pos])
                                nc.vector.tensor_add(qrT[:, sl], t1[:], t2[:])

                            def denom_dve(probs_p):
                                """bf16 pairwise-add tree over sk tiles."""
                                p8 = phC.tile([128, 2, 8, 512], BF16,
                                              tag="p8", bufs=1, name="p8")
                                for h in range(2):
                                    nc.vector.tensor_add(
                                        p8[:, h, :, :],
                                        probs_p[:, 0:8, h, :],
                                        probs_p[:, 8:16, h, :])
                                p4 = phC.tile([128, 2, 4, 512], BF16,
                                              tag="p4", bufs=1, name="p4")
                                for h in range(2):
                                    nc.vector.tensor_add(
                                        p4[:, h, :, :],
                                        p8[:, h, 0:4, :],
                                        p8[:, h, 4:8, :])
                                p2 = phC.tile([128, 2, 2, 512], BF16,
                                              tag="p2", bufs=1, name="p2")
                                for h in range(2):
                                    nc.vector.tensor_add(
                                        p2[:, h, :, :],
                                        p4[:, h, 0:2, :],
                                        p4[:, h, 2:4, :])
                                return p2

                            def denom_pe(p4, pe_bcast=False):
                                """column-sum + reciprocal + broadcast."""
                                dsum = psC.tile([128, 2, 512], F32,
                                                tag="dsum", bufs=1,
                                                name="dsum")
                                for h in range(2):
                                    for i in range(2):
                                        nc.tensor.matmul(
                                            dsum[0:1, h, :], ones_col[:],
                                            p4[:, h, i, :],
                                            start=(i == 0), stop=(i == 1))
                                rec = phC.tile([1, 2, 512], F32, tag="rec",
                                               bufs=1, name="rec")
                                recb = phC.tile([128, 2, 512], F32,
                                                tag="recb", bufs=1,
                                                name="recb")
                                nc.vector.reciprocal_approx_accurate(
                                    out=rec[0:1, :, :],
                                    in_=dsum[0:1, :, :],
                                    scratch=recb[0:1, :, :])
                                if pe_bcast:
                                    for h in range(2):
                                        nc.tensor.matmul(dsum[:, h, :],
                                                         ones_f32[:],
                                                         rec[0:1, h, :],
                                                         start=True,
                                                         stop=True)
                                    nc.vector.tensor_copy(recb[:], dsum[:])
                                else:
                                    nc.gpsimd.partition_broadcast(
                                        recb[:], rec[0:1, :, :])
                                return recb

                            def finish_stg(prev, psvs, recb):
                                pg, pb, probs_p = prev
                                for h in range(2):
                                    stg = phC.tile([128, 512], BF16,
                                                   tag=f"stg{h}", bufs=1,
                                                   name=f"stg{h}")
                                    nc.vector.tensor_mul(stg[:], psvs[h][:],
                                                         recb[:, h, :])
                                    nc.sync.dma_start(
                                        out=a2a_i[pg,
                                                  h * 128:(h + 1) * 128, :],
                                        in_=stg[:])

                            pass2(0)
                            pass2(1)
                            # software pipeline: block g's scores+exp stream
                            # carries block g-1's PV + denominator matmuls so
                            # the tensor engine never drains behind the
                            # scalar-engine exp chain. Column-sums for g-1 are
                            # issued right after the second score group so the
                            # reciprocal/broadcast chain retires mid-iteration.
                            prev = None
                            for b in range(2):
                                for sqb in range(4):
                                    g = b * 4 + sqb
                                    qsl = slice(b * S + sqb * 512,
                                                b * S + sqb * 512 + 512)
                                    probs = phC.tile([128, 16, 2, 512], BF16,
                                                     tag="probs", bufs=2,
                                                     name="probs")
                                    if prev is not None:
                                        p4p = denom_dve(prev[2])
                                        psvs = [psC.tile([128, 512], F32,
                                                         tag=f"psv{h}",
                                                         bufs=1,
                                                         name=f"psv{h}")
                                                for h in range(2)]
                                    recb = None
                                    for skt in range(16):
                                        ps2 = psC.tile([128, 2, 512], F32,
                                                       tag="ps2", bufs=2,
                                                       name="ps2")
                                        ksl = slice(b * S + skt * 128,
                                                    b * S + skt * 128 + 128)
                                        for h in range(2):
                                            nc.tensor.matmul(
                                                ps2[:, h, :],
                                                kcT[h][:, ksl],
                                                qcT[h][:, qsl],
                                                start=True, stop=False)
                                        for h in range(2):
                                            hr = slice(h * 64, h * 64 + 64)
                                            nc.tensor.matmul(
                                                ps2[:, h, :],
                                                krT[hr, ksl],
                                                qrT[hr, qsl],
                                                start=False, stop=True)
                                        nc.scalar.activation(
                                            probs[:, skt, :, :], ps2[:], EXP)
                                        if prev is not None:
                                            pg, pb, probs_p = prev
                                            for h in range(2):
                                                nc.tensor.matmul(
                                                    psvs[h][:],
                                                    v_sb[:, pb * 16 + skt,
                                                         h * 128:
                                                         (h + 1) * 128],
                                                    probs_p[:, skt, h, :],
                                                    start=(skt == 0),
                                                    stop=(skt == 15))
                                            if skt == 5:
                                                recb = denom_pe(p4p)
                                    if prev is not None:
                                        finish_stg(prev, psvs, recb)
                                    pass2(g + 2)
                                    prev = (g, b, probs)
                            # drain the last block; thin per-pair adds let
                            # the reduction overlap the tail of the exp chain
                            pg, pb, probs_p = prev
                            p8d = phC.tile([128, 2, 8, 512], BF16,
                                           tag="p8", bufs=1, name="p8d")
                            for i in range(8):
                                for h in range(2):
                                    nc.vector.tensor_add(
                                        p8d[:, h, i, :],
                                        probs_p[:, i, h, :],
                                        probs_p[:, i + 8, h, :])
                            p4d = phC.tile([128, 2, 4, 512], BF16,
                                           tag="p4", bufs=1, name="p4d")
                            for h in range(2):
                                nc.vector.tensor_add(p4d[:, h, :, :],
                                                     p8d[:, h, 0:4, :],
                                                     p8d[:, h, 4:8, :])
                            p4p = phC.tile([128, 2, 2, 512], BF16,
                                           tag="p2", bufs=1, name="p2d")
                            for h in range(2):
                                nc.vector.tensor_add(p4p[:, h, :, :],
                                                     p4d[:, h, 0:2, :],
                                                     p4d[:, h, 2:4, :])
                            psvs = [psC.tile([128, 512], F32, tag=f"psv{h}",
                                             bufs=1, name=f"psv{h}")
                                    for h in range(2)]
                            for skt in range(16):
                                for h in range(2):
                                    nc.tensor.matmul(
                                        psvs[h][:],
                                        v_sb[:, pb * 16 + skt,
                                             h * 128:(h + 1) * 128],
                                        probs_p[:, skt, h, :],
                                        start=(skt == 0), stop=(skt == 15))
                            finish_stg(prev, psvs,
                                       denom_pe(p4p, pe_bcast=True))
                            nc.gpsimd.collective_compute(
                                "AllToAll", mybir.AluOpType.bypass,
                                replica_groups=groups,
                                ins=[a2a_i.opt()], outs=[a2a_o.opt()])

                        # ============== Phase D: out projection ==============
                        with tc.tile_pool(name="phD", bufs=1) as phD, \
                             tc.tile_pool(name="phDw", bufs=2) as phDw, \
                             tc.tile_pool(name="phDo", bufs=3) as phDo, \
                             tc.tile_pool(name="psD", bufs=2,
                                          space="PSUM") as psD:
                            wos0 = []
                            for dht in range(16):
                                wo = phDw0.tile([128, 512], BF16,
                                                tag=f"wo0{dht}",
                                                name=f"wo0{dht}")
                                nc.scalar.dma_start(
                                    out=wo[:],
                                    in_=woT[dht * 128:(dht + 1) * 128,
                                            0:512])
                                wos0.append(wo)
                            psj2 = psD.tile([128, 512], F32, tag="junk",
                                            bufs=1, name="psj2")
                            for _ in range(180):
                                nc.tensor.matmul(psj2[:],
                                                 wos0[0][:, 0:128],
                                                 wos0[1][:],
                                                 start=True, stop=True)
                            csl = []
                            for dht in range(16):
                                cf = phD.tile([128, S_LOC], BF16,
                                              tag=f"cf{dht}", name=f"cf{dht}")
                                nc.sync.dma_start(
                                    out=cf[:],
                                    in_=a2a_o[dht // 2,
                                              (dht % 2) * 128:
                                              (dht % 2) * 128 + 128, :])
                                csl.append(cf)
                            for ot in range(4):
                                osl = slice(ot * 512, (ot + 1) * 512)
                                if ot == 0:
                                    wos = wos0
                                else:
                                    wos = []
                                    for dht in range(16):
                                        wo = phDw.tile([128, 512], BF16,
                                                       tag=f"wo{dht}",
                                                       name=f"wo{dht}")
                                        nc.sync.dma_start(
                                            out=wo[:],
                                            in_=woT[dht * 128:(dht + 1) * 128,
                                                    osl])
                                        wos.append(wo)
                                for ssub in range(4):
                                    pso = psD.tile([128, 512], F32, tag="psO",
                                                   name="pso")
                                    ssl = slice(ssub * 128, (ssub + 1) * 128)
                                    for dht in range(16):
                                        nc.tensor.matmul(pso[:],
                                                         csl[dht][:, ssl],
                                                         wos[dht][:],
                                                         start=(dht == 0),
                                                         stop=(dht == 15))
                                    osb = phDo.tile([128, 512], F32,
                                                    tag="osb", name="osb")
                                    nc.vector.tensor_copy(osb[:], pso[:])
                                    nc.sync.dma_start(out=out_sl[ssl, osl],
                                                      in_=osb[:])

    nc.compile()
    return nc


def _host_prep(inputs):
    """Build per-core input maps from the full problem inputs."""
    x = np.asarray(inputs["x"], np.float32)
    xT = np.ascontiguousarray(x.reshape(BS, HID).T)            # [HID, BS]
    wdkvT = np.ascontiguousarray(np.asarray(inputs["d_kv_w"], np.float32).T.astype(BF))
    wdqT = np.ascontiguousarray(np.asarray(inputs["d_q_w"], np.float32).T.astype(BF))
    bdkv = np.asarray(inputs["d_kv_b"], np.float32).reshape(4, 128, 1)
    bdq = np.asarray(inputs["d_q_b"], np.float32).reshape(4, 128, 1)

    uk3 = np.asarray(inputs["u_k_w"], np.float32).reshape(H, Dh, C)
    uq3 = np.asarray(inputs["u_q_w"], np.float32).reshape(H, Dh, C) * SCALE
    uv3 = np.asarray(inputs["u_v_w"], np.float32).reshape(H, Dh, C)
    buk2 = np.asarray(inputs["u_k_b"], np.float32).reshape(H, Dh)
    buq2 = np.asarray(inputs["u_q_b"], np.float32).reshape(H, Dh) * SCALE
    qr3 = np.asarray(inputs["qr_w"], np.float32).reshape(H, Dr, C)

    # rope tables (positions 0..S-1)
    i32 = np.arange(32, dtype=np.float32)
    inv_freq = (10000.0 ** (-(2.0 * i32) / Dr)).astype(np.float32)  # [32]
    pos = np.arange(S, dtype=np.float32)
    ang = pos[None, :] * inv_freq[:, None]                     # [32, S]
    cos, sin = np.cos(ang), np.sin(ang)
    c1 = np.concatenate([cos, sin, cos, sin], 0).astype(np.float32)
    c2 = np.concatenate([-sin, cos, -sin, cos], 0).astype(np.float32)

    woT = np.ascontiguousarray(np.asarray(inputs["out_w"], np.float32).T.astype(BF))

    in_maps = []
    for j in range(NCORES):
        hs = [2 * j, 2 * j + 1]
        ukT_l = uk3[hs].transpose(2, 0, 1).reshape(C, 256)
        uqT_l = uq3[hs].transpose(2, 0, 1).reshape(C, 256)
        uvT_l = uv3[hs].transpose(2, 0, 1).reshape(C, 256)
        we = [qr3[h, 0::2, :] for h in hs]    # [32, C] each
        wo = [qr3[h, 1::2, :] for h in hs]
        wrA = np.concatenate([we[0], we[0], we[1], we[1]], 0).T  # [C, 128]
        wrB = np.concatenate([wo[0], wo[0], wo[1], wo[1]], 0).T
        in_maps.append({
            "xT_loc": np.ascontiguousarray(
                xT[:, j * S_LOC:(j + 1) * S_LOC]).astype(BF),
            "wdkvT": wdkvT, "wdqT": wdqT, "bdkv": bdkv, "bdq": bdq,
            "ukT": np.ascontiguousarray(ukT_l.astype(BF)),
            "uqT": np.ascontiguousarray(uqT_l.astype(BF)),
            "uvT": np.ascontiguousarray(uvT_l.astype(BF)),
            "buk": buk2[hs].reshape(2, 128, 1).copy(),
            "buq": buq2[hs].reshape(2, 128, 1).copy(),
            "wrAq": np.ascontiguousarray((wrA * SCALE).astype(BF)),
            "wrBq": np.ascontiguousarray((wrB * SCALE).astype(BF)),
            "wrAk": np.ascontiguousarray(wrA.astype(BF)),
            "wrBk": np.ascontiguousarray(wrB.astype(BF)),
            "c1": c1.astype(np.float16), "c2": c2.astype(np.float16),
            "woT": woT,
        })
    return in_maps


def kernel(**inputs):
    if "nc" not in _CACHE:
        _CACHE["nc"] = _build_program()
    nc = _CACHE["nc"]
    in_maps = _host_prep(inputs)
    res = run_bass_kernel_spmd(nc, in_maps, list(range(NCORES)))
    out = np.concatenate([res.results[j]["out_slice"] for j in range(NCORES)], 0)
    out = out + np.asarray(inputs["out_b"], np.float32)[None, :]
    return out.reshape(B, S, HID)
